# revision 1
# baseline (speedup 1.0000x reference)
"""GQA causal attention block (B=2, L=2048, d_model=2048, 32 Q heads / 8 KV heads)
on 8 TRN2 NeuronCores.

Sharding: 8-way tensor parallel over heads, batch-paired. Core c owns q-heads
[4c, 4c+4) and kv-head c FOR BOTH batches.

Layouts:
  - qT (bf16): 4 tiles [128, L] = head-pair x batch: tile (t, b) holds heads
    2t (partitions 0:64) and 2t+1 (64:128) of batch b, feature-major.
  - kT (bf16): per batch a [128, L] tile with the kv head DUPLICATED in both
    partition halves, so scores for odd heads read lhsT/rhs at matching base 64.
  - v_aug (bf16): per key-block [128, 130]: cols 0:65 = batch0 (v | ones),
    65:130 = batch1 (v | ones).

Per head+batch, causal attention runs in transposed layout: scores^T = matmul
(kT stationary, qT moving), exp on ScalarE straight out of PSUM into bf16 P
tiles (unnormalized), AV matmul against V-with-ones-column emits both attn^T
and the softmax denominator. One 8-core AllToAll switches head-sharding ->
sequence-sharding; each core then normalizes (reciprocal + partition-broadcast
DMA + DVE multiply) and runs o_proj (fp32r) against the full Wo for its 512
output rows. The host just stacks rows.
"""

import os
import sys
import math

os.environ.setdefault("MYCRO_LOCAL_CACHE", "1")
for _p in ("/opt/trn_rl_repo",):
    if os.path.isdir(_p) and _p not in sys.path:
        sys.path.insert(0, _p)

import numpy as np

import concourse.bass as bass
import concourse.bacc as bacc
import concourse.mybir as mybir
import concourse.tile as tile
from concourse.bass_utils import run_bass_kernel_spmd
from concourse.masks import make_identity

F32 = mybir.dt.float32
F32R = mybir.dt.float32r
BF16 = mybir.dt.bfloat16
Exp = mybir.ActivationFunctionType.Exp

D = 2048          # d_model
L = 2048          # sequence length
DH = 64           # head dim
B = 2             # batch
NCORES = 8
NH_L = 4          # local q heads per core (per batch)
QF = NH_L * DH    # 256 local q features per batch
LC1 = 256         # phase-1 l-chunk (moving dim)
NLC1 = L // LC1   # 8
LC = 512          # attention l-tile
NLC = L // LC     # 4
NB = L // 128     # 16 key blocks of 128
SH = QF + NH_L    # 260 rows per A2A shard (4 heads x 64 + 4 denoms)
SCALE = 1.0 / math.sqrt(DH)

_CACHE = {}


def _mmr(nc, out, lhsT, rhs, **kw):
    """float32r matmul (TF32-ish). Operands must come from f32r-producing
    instructions (gpsimd casting DMA / DVE ops)."""
    nc.tensor.matmul(out, lhsT, rhs, **kw)


def _build_nc():
    nc = bacc.Bacc(
        "TRN2",
        target_bir_lowering=False,
        debug=False,
        enable_asserts=False,
        num_devices=NCORES,
    )
    xT0 = nc.dram_tensor("xT0", [D, L], F32, kind="ExternalInput")
    xT1 = nc.dram_tensor("xT1", [D, L], F32, kind="ExternalInput")
    wqT = nc.dram_tensor("wqT", [D, QF], F32, kind="ExternalInput")
    wkT = nc.dram_tensor("wkT", [D, DH], F32, kind="ExternalInput")
    wvT = nc.dram_tensor("wvT", [D, DH], F32, kind="ExternalInput")
    woT = nc.dram_tensor("woT", [D, D], F32, kind="ExternalInput")
    y = nc.dram_tensor("y", [LC, D], F32, kind="ExternalOutput")

    with tile.TileContext(nc) as tc:
        with tc.tile_pool(name="dram", bufs=1, space="DRAM") as dram:
            bin_ = dram.tile([NCORES * SH, LC], F32, name="bounce_in")
            bout = dram.tile([NCORES * SH, LC], F32, name="bounce_out")
            rdram = dram.tile([32, 512], F32, name="rdram")

            with tc.tile_pool(name="const", bufs=1) as const:
                ident = const.tile([128, 128], F32, name="ident")
                make_identity(nc, ident)

                with tc.tile_pool(name="pers", bufs=1) as pers:
                    # q: [pair t][batch b] -> [128, L] bf16 (heads 2t | 2t+1)
                    qT = [
                        [
                            pers.tile([128, L], BF16, name=f"qT{t}{b}")
                            for b in range(2)
                        ]
                        for t in range(2)
                    ]
                    # kT per batch, kv head duplicated in both halves
                    kT = [pers.tile([128, L], BF16, name=f"kT{b}") for b in range(2)]
                    vaug = pers.tile([128, NB * 130], BF16, name="vaug")
                    va = vaug.rearrange("p (b c) -> p b c", c=130)
                    nc.gpsimd.memset(va[:, :, 64:65], 1.0)
                    nc.gpsimd.memset(va[:, :, 129:130], 1.0)

                    _phase1_qkv(nc, tc, xT0, xT1, wqT, wkT, wvT, qT, kT, va, ident)
                    _phase2_attn(nc, tc, qT, kT, va, bin_)
                    nc.gpsimd.collective_compute(
                        "AllToAll",
                        mybir.AluOpType.bypass,
                        ins=[bin_.opt()],
                        outs=[bout.opt()],
                        replica_groups=[list(range(NCORES))],
                    )
                    _phase4_oproj(nc, tc, bout, woT, rdram, y)
    nc.finalize()  # bacc: register allocation, ACT table loads, etc.
    return nc


def _phase1_qkv(nc, tc, xT0, xT1, wqT, wkT, wvT, qT, kT, va, ident):
    """Projections. q: one M=128 fp32r matmul per (head-pair, batch, db).
    k/v: M=64 per batch at psum base 0; the partition-64 halves of kT are
    filled via a bf16 staging tile + SBUF->SBUF DMA (matmul psum dst must
    start at partition 0)."""
    with (
        tc.tile_pool(name="w1", bufs=1) as wpool,
        tc.tile_pool(name="xc", bufs=2) as xpool,
        tc.tile_pool(name="vt", bufs=2) as vtpool,
        tc.tile_pool(name="p1", bufs=1, space="PSUM") as p1,
    ):
        wq_sb = wpool.tile([128, 16 * QF], F32R, name="wq_sb")
        wk_sb = wpool.tile([128, 16 * DH], F32R, name="wk_sb")
        wv_sb = wpool.tile([128, 16 * DH], F32R, name="wv_sb")
        for w_sb, w_dram, fw in ((wq_sb, wqT, QF), (wk_sb, wkT, DH), (wv_sb, wvT, DH)):
            nc.gpsimd.dma_start(
                w_sb.rearrange("p (b f) -> p b f", f=fw),
                w_dram.rearrange("(b p) f -> p b f", p=128),
            )

        for lc in range(NLC1):
            x0 = xpool.tile([128, 16 * LC1], F32R, name="x0", tag="x0")
            x1 = xpool.tile([128, 16 * LC1], F32R, name="x1", tag="x1")
            for xt, xdram in ((x0, xT0), (x1, xT1)):
                nc.gpsimd.dma_start(
                    xt.rearrange("p (b l) -> p b l", l=LC1),
                    xdram[:, lc * LC1 : (lc + 1) * LC1].rearrange(
                        "(b p) l -> p b l", p=128
                    ),
                )
            cols = slice(lc * LC1, (lc + 1) * LC1)
            # pass A: q-pair0 (both batches) + k (both); pass B: q-pair1 + v
            for grp in range(2):
                aq = [
                    p1.tile([128, LC1], F32, name=f"aq{b}", tag=f"aq{b}")
                    for b in range(2)
                ]
                akv = [
                    p1.tile([64, LC1], F32, name=f"akv{b}", tag=f"akv{b}")
                    for b in range(2)
                ]
                for db in range(16):
                    rx = (
                        x0[:, db * LC1 : (db + 1) * LC1],
                        x1[:, db * LC1 : (db + 1) * LC1],
                    )
                    st = dict(start=(db == 0), stop=(db == 15))
                    wjp = wq_sb[:, db * QF + grp * 128 : db * QF + (grp + 1) * 128]
                    wkv = wk_sb if grp == 0 else wv_sb
                    wb = wkv[:, db * DH : (db + 1) * DH]
                    for b in range(2):
                        _mmr(nc, aq[b][:, :], wjp, rx[b], **st)
                        _mmr(nc, akv[b][:, :], wb, rx[b], **st)
                for b in range(2):
                    # q copyback: psum f32 -> bf16, partitions already paired
                    nc.scalar.copy(qT[grp][b][:, cols], aq[b][:, :])
                if grp == 0:
                    for b in range(2):
                        nc.scalar.copy(kT[b][0:64, cols], akv[b][:, :])
                        stk = vtpool.tile([64, LC1], BF16, name="stk", tag=f"stk{b}")
                        nc.scalar.copy(stk[:, :], akv[b][:, :])
                        nc.sync.dma_start(kT[b][64:128, cols], stk[:, :])
                else:
                    vt0 = vtpool.tile([64, LC1], F32, name="vt0", tag="vt0")
                    vt1 = vtpool.tile([64, LC1], F32, name="vt1", tag="vt1")
                    nc.scalar.copy(vt0[:, :], akv[0][:, :])
                    nc.scalar.copy(vt1[:, :], akv[1][:, :])
                    for s in range(LC1 // 128):
                        beta = (lc * LC1) // 128 + s
                        tp = p1.tile([128, 128], F32, name="tp", tag="tp", bufs=2)
                        nc.tensor.matmul(
                            tp[:, 0:64],
                            vt0[:, s * 128 : (s + 1) * 128],
                            ident[0:64, 0:64],
                            is_transpose=True,
                        )
                        nc.tensor.matmul(
                            tp[:, 64:128],
                            vt1[:, s * 128 : (s + 1) * 128],
                            ident[0:64, 0:64],
                            is_transpose=True,
                            skip_group_check=True,
                        )
                        nc.scalar.copy(va[:, beta, 0:64], tp[:, 0:64])
                        nc.scalar.copy(va[:, beta, 65:129], tp[:, 64:128])


def _phase2_attn(nc, tc, qT, kT, va, bin_):
    """Transposed-scores causal attention (bf16 QK^T and AV)."""
    with (
        tc.tile_pool(name="p2s", bufs=1, space="PSUM") as scp,
        tc.tile_pool(name="p2o", bufs=2, space="PSUM") as ovp,
        tc.tile_pool(name="pbuf", bufs=1) as pbp,
        tc.tile_pool(name="stg", bufs=3) as stp,
    ):
        for tau in range(NLC):
            for j in range(4):  # local q head
                t, hh = divmod(j, 2)
                po = 64 * hh  # partition base inside the pair tile
                nb = 4 * tau + 4
                pa = pbp.tile([128, NB * 512], BF16, name="pa", tag="pa")
                pb = pbp.tile([128, NB * 512], BF16, name="pb", tag="pb")
                lcols = slice(tau * LC, (tau + 1) * LC)
                qa = qT[t][0][po : po + 64, lcols]
                qb = qT[t][1][po : po + 64, lcols]

                # full (unmasked) strips, two key-blocks per exp call
                for b0 in range(0, 4 * tau, 2):
                    for hi, (q, kTb, P) in enumerate(
                        ((qa, kT[0], pa), (qb, kT[1], pb))
                    ):
                        sc = scp.tile([128, 1024], F32, name="sc", tag=f"sc{hi}")
                        nc.tensor.matmul(
                            sc[:, 0:512],
                            kTb[po : po + 64, b0 * 128 : (b0 + 1) * 128],
                            q,
                        )
                        nc.tensor.matmul(
                            sc[:, 512:1024],
                            kTb[po : po + 64, (b0 + 1) * 128 : (b0 + 2) * 128],
                            q,
                        )
                        nc.scalar.activation(
                            P[:, b0 * 512 : (b0 + 2) * 512], sc[:, 0:1024], Exp
                        )
                # diagonal strips (block-level causal masking)
                for dj in range(4):
                    beta = 4 * tau + dj
                    for hi, (q, kTb, P) in enumerate(
                        ((qa, kT[0], pa), (qb, kT[1], pb))
                    ):
                        sc = scp.tile([128, 1024], F32, name="sc", tag=f"sc{hi}")
                        nc.tensor.matmul(
                            sc[:, 0:512],
                            kTb[po : po + 64, beta * 128 : (beta + 1) * 128],
                            q,
                        )
                        if dj > 0:
                            nc.gpsimd.memset(
                                P[:, beta * 512 : beta * 512 + dj * 128], 0.0
                            )
                        nc.scalar.activation(
                            P[:, beta * 512 + dj * 128 : (beta + 1) * 512],
                            sc[:, dj * 128 : 512],
                            Exp,
                        )
                        dg = P[:, beta * 512 + dj * 128 : beta * 512 + (dj + 1) * 128]
                        nc.gpsimd.affine_select(
                            out=dg,
                            in_=dg,
                            compare_op=mybir.AluOpType.is_ge,
                            fill=0.0,
                            base=0,
                            pattern=[[1, 128]],
                            channel_multiplier=-1,
                        )
                # AV (+denominator via the ones column of v_aug)
                oa = ovp.tile([128, 512], F32, name="oa", tag="oa")
                ob = ovp.tile([128, 512], F32, name="ob", tag="ob")
                for b in range(nb):
                    st = dict(start=(b == 0), stop=(b == nb - 1))
                    nc.tensor.matmul(
                        oa[0:65, :], va[:, b, 0:65],
                        pa[:, b * 512 : (b + 1) * 512], **st,
                    )
                    nc.tensor.matmul(
                        ob[0:65, :], va[:, b, 65:130],
                        pb[:, b * 512 : (b + 1) * 512], **st,
                    )
                # stage attn rows + denominators -> A2A bounce buffer.
                # dest shard for (batch bb, l-block tau) is 4*bb + tau;
                # row inside shard = 64*j (+256..259 for denoms).
                st1 = stp.tile([128, 512], F32, name="st1", tag="st1")
                nc.scalar.copy(st1[0:64, :], oa[0:64, :])
                nc.scalar.copy(st1[64:128, :], ob[0:64, :])
                for bb, half in ((0, st1[0:64, :]), (1, st1[64:128, :])):
                    sh = SH * (4 * bb + tau)
                    nc.sync.dma_start(
                        bin_[sh + 64 * j : sh + 64 * (j + 1), :], half
                    )
                ds = stp.tile([128, 1024], F32, name="ds", tag="ds")
                nc.vector.tensor_copy(ds[64:65, 0:512], oa[64:65, :])
                nc.vector.tensor_copy(ds[64:65, 512:1024], ob[64:65, :])
                for bb in range(2):
                    sh = SH * (4 * bb + tau)
                    nc.sync.dma_start(
                        bin_[sh + QF + j : sh + QF + j + 1, :],
                        ds[64:65, 512 * bb : 512 * bb + 512],
                    )


def _phase4_oproj(nc, tc, bout, woT, rdram, y):
    """Normalize (divide by softmax denominators) and run o_proj for this
    core's 512 sequence rows against the full Wo."""
    with (
        tc.tile_pool(name="an", bufs=1) as anp,
        tc.tile_pool(name="wo", bufs=2) as wop,
        tc.tile_pool(name="den", bufs=1) as denp,
        tc.tile_pool(name="ysb", bufs=2) as yp,
        tc.tile_pool(name="p4y", bufs=4, space="PSUM") as eyp,
    ):
        # denominators: shard c rows 256:260 = heads 4c..4c+3
        dall = denp.tile([32, 512], F32, name="dall")
        for c in range(NCORES):
            nc.sync.dma_start(
                dall[4 * c : 4 * (c + 1), :],
                bout[SH * c + QF : SH * c + QF + NH_L, :],
            )
        rall_f = denp.tile([32, 512], F32, name="rall_f")
        nc.vector.reciprocal(rall_f[:, :], dall[:, :])
        nc.sync.dma_start(rdram[:, :], rall_f[:, :])

        ans = []
        for ft in range(16):
            c, half = divmod(ft, 2)
            au = anp.tile([128, 512], F32, name=f"au{ft}", tag=f"au{ft}")
            nc.sync.dma_start(
                au[:, :],
                bout[SH * c + 128 * half : SH * c + 128 * (half + 1), :],
            )
            hA = 2 * ft
            hB = 2 * ft + 1
            dv = anp.tile([128, 512], F32, name="dv", tag="dv", bufs=2)
            nc.sync.dma_start(
                dv[0:64, :], rdram[hA : hA + 1, :].partition_broadcast(64)
            )
            nc.sync.dma_start(
                dv[64:128, :], rdram[hB : hB + 1, :].partition_broadcast(64)
            )
            an = anp.tile([128, 512], F32R, name=f"an{ft}", tag=f"an{ft}")
            nc.vector.tensor_mul(an[:, :], au[:, :], dv[:, :])
            ans.append(an)

        for dc in range(4):
            wo_t = wop.tile([128, 16 * 512], F32R, name="wo_t", tag="wo")
            nc.gpsimd.dma_start(
                wo_t.rearrange("p (b d) -> p b d", d=512),
                woT[:, dc * 512 : (dc + 1) * 512].rearrange("(b p) d -> p b d", p=128),
            )
            for m in range(4):
                yps = eyp.tile([128, 512], F32, name="yps", tag="yps")
                for k in range(16):
                    _mmr(
                        nc, yps[:, :],
                        ans[k][:, m * 128 : (m + 1) * 128],
                        wo_t[:, k * 512 : (k + 1) * 512],
                        start=(k == 0), stop=(k == 15),
                    )
                ysb = yp.tile([128, 512], F32, name="ysb", tag="ysb")
                nc.scalar.copy(ysb[:, :], yps[:, :])
                nc.sync.dma_start(
                    y[m * 128 : (m + 1) * 128, dc * 512 : (dc + 1) * 512], ysb[:, :]
                )


def _get_nc():
    if "nc" not in _CACHE:
        _CACHE["nc"] = _build_nc()
    return _CACHE["nc"]


LAST_EXEC_NS = None


def kernel(x, Wq, Wk, Wv, Wo):
    global LAST_EXEC_NS
    x = np.asarray(x, dtype=np.float32)
    Wq = np.asarray(Wq, dtype=np.float32)
    Wk = np.asarray(Wk, dtype=np.float32)
    Wv = np.asarray(Wv, dtype=np.float32)
    Wo = np.asarray(Wo, dtype=np.float32)

    xT0 = np.ascontiguousarray(x[0].T)
    xT1 = np.ascontiguousarray(x[1].T)
    woT = np.ascontiguousarray(Wo.T)

    in_maps = []
    for c in range(NCORES):
        wqT_c = np.ascontiguousarray((SCALE * Wq[QF * c : QF * (c + 1), :]).T)
        wkT_c = np.ascontiguousarray(Wk[DH * c : DH * (c + 1), :].T)
        wvT_c = np.ascontiguousarray(Wv[DH * c : DH * (c + 1), :].T)
        in_maps.append(
            {
                "xT0": xT0,
                "xT1": xT1,
                "wqT": wqT_c,
                "wkT": wkT_c,
                "wvT": wvT_c,
                "woT": woT,
            }
        )

    nc = _get_nc()
    res = run_bass_kernel_spmd(nc, in_maps, core_ids=list(range(NCORES)))
    LAST_EXEC_NS = getattr(res, "exec_time_ns", None)

    out = np.empty((B, L, D), dtype=np.float32)
    for c in range(NCORES):
        b, g = divmod(c, 4)
        out[b, 512 * g : 512 * (g + 1), :] = res.results[c]["y"]
    return out



# revision 8
# speedup vs baseline: 1.3651x; 1.3651x over previous
"""GQA causal attention block (B=2, L=2048, d_model=2048, 32 Q heads / 8 KV heads)
on 8 TRN2 NeuronCores.

Sharding: 8-way tensor parallel over heads. Core c owns q-heads [4c, 4c+4) and
kv-head c for both batches. After attention, an AllToAll switches head-sharding
-> sequence-sharding (core c = batch c//4, seq block c%4 of 512); each core then
normalizes and runs o_proj against the full Wo for its 512 rows.

All matmuls run in bf16 (fp32 PSUM accumulation). Layouts:
  - qh[h] (bf16): per local head a [128, L] tile: batch0 features in partitions
    0:64, batch1 in 64:128 (built by column-tiled projection matmuls that run
    concurrently on the PE array).
  - kbT (bf16): [128, L]: batch0 kv-head in partitions 0:64, batch1 in 64:128,
    so transposed scores for the two batches run as concurrent row-tiled
    matmuls (row groups 0-1 and 2-3).
  - va (bf16): per key-block [128, 130]: cols 0:64 = v(b0), col 64 = ones,
    cols 65:129 = v(b1), col 129 = ones. The ones column makes the AV matmul
    emit the softmax denominator for free.
  - P (bf16, rolling blocks): per key-block [128, 2048] =
    [h0b0 | h1b0 | h0b1 | h1b1] x 512 queries, so AV streams 1024-wide.

Phase2 trims causal-diagonal blocks (scores/exp only on the valid query range;
the invalid P region is zeroed so AV can stay 1024-wide), and applies the
within-block triangle via gpsimd affine_select.
"""

import os
import sys
import math

os.environ.setdefault("MYCRO_LOCAL_CACHE", "1")
for _p in ("/opt/trn_rl_repo",):
    if os.path.isdir(_p) and _p not in sys.path:
        sys.path.insert(0, _p)

import numpy as np
import ml_dtypes

import concourse.bass as bass
import concourse.bacc as bacc
import concourse.mybir as mybir
import concourse.tile as tile
from concourse.bass_utils import run_bass_kernel_spmd
from concourse.masks import make_identity

F32 = mybir.dt.float32
BF16 = mybir.dt.bfloat16
Exp = mybir.ActivationFunctionType.Exp

D = 2048          # d_model
L = 2048          # sequence length
DH = 64           # head dim
B = 2             # batch
NCORES = 8
NH_L = 4          # local q heads per core (per batch)
QF = NH_L * DH    # 256 local q features per batch
LC1 = 512         # phase-1 token chunk
NLC1 = L // LC1   # 4
LT = 512          # attention query tile (per tau)
NT = L // LT      # 4
NB = L // 128     # 16 key blocks of 128
SH = NH_L * (DH + 1)  # 260 rows per A2A shard (4 x (64 attn + 1 denom))
SCALE = 1.0 / math.sqrt(DH)

_CACHE = {}


def _build_nc():
    nc = bacc.Bacc(
        "TRN2",
        target_bir_lowering=False,
        debug=False,
        enable_asserts=False,
        num_devices=NCORES,
    )
    # host-prepped layouts (see kernel() below)
    xh0 = nc.dram_tensor("xh0", [NLC1 * 128, 16 * LC1], BF16, kind="ExternalInput")
    xh1 = nc.dram_tensor("xh1", [NLC1 * 128, 16 * LC1], BF16, kind="ExternalInput")
    wqh = nc.dram_tensor("wqh", [128, 16 * QF], BF16, kind="ExternalInput")
    wkh = nc.dram_tensor("wkh", [128, 16 * DH], BF16, kind="ExternalInput")
    wvh = nc.dram_tensor("wvh", [128, 16 * DH], BF16, kind="ExternalInput")
    woh = nc.dram_tensor("woh", [D, D], BF16, kind="ExternalInput")  # Wo.T
    y = nc.dram_tensor("y", [LT, D], F32, kind="ExternalOutput")

    with tile.TileContext(nc) as tc:
        with tc.tile_pool(name="dram", bufs=1, space="DRAM") as dram:
            bin_ = dram.tile([NCORES * SH, LT], BF16, name="bounce_in")
            bout = dram.tile([NCORES * SH, LT], BF16, name="bounce_out")

            with tc.tile_pool(name="const", bufs=1) as const:
                ident = const.tile([128, 128], BF16, name="ident")
                make_identity(nc, ident)

                with tc.tile_pool(name="pers", bufs=1) as pers:
                    qh = [pers.tile([128, L], BF16, name=f"qh{h}") for h in range(4)]
                    kbT = pers.tile([128, L], BF16, name="kbT")
                    vaug = pers.tile([128, NB * 130], BF16, name="vaug")
                    va = vaug.rearrange("p (b c) -> p b c", c=130)
                    nc.gpsimd.memset(va[:, :, 64:65], 1.0)
                    nc.gpsimd.memset(va[:, :, 129:130], 1.0)

                    _phase1_qkv(nc, tc, xh0, xh1, wqh, wkh, wvh, qh, kbT, va, ident)
                    with tc.tile_pool(name="wo", bufs=1) as wop:
                        wo_sb = wop.tile([128, 16 * D], BF16, name="wo_sb")
                        # prefetch the full Wo during phase 2
                        nc.gpsimd.dma_start(
                            wo_sb.rearrange("p (k d) -> p k d", d=D),
                            woh.rearrange("(k p) d -> p k d", p=128),
                        )
                        _phase2_attn(nc, tc, qh, kbT, va, bin_)
                        nc.gpsimd.collective_compute(
                            "AllToAll",
                            mybir.AluOpType.bypass,
                            ins=[bin_.opt()],
                            outs=[bout.opt()],
                            replica_groups=[list(range(NCORES))],
                        )
                        _phase4_oproj(nc, tc, bout, wo_sb, y)
    nc.finalize()
    return nc


def _phase1_qkv(nc, tc, xh0, xh1, wqh, wkh, wvh, qh, kbT, va, ident):
    """Projections, column-tiled over the two batches: for each output head
    (or kv head) the b0 matmul writes psum partitions 0:64 (array col groups
    0-1) and the b1 matmul writes partitions 64:128 (groups 2-3); the two run
    concurrently on the PE array."""
    with (
        tc.tile_pool(name="w1", bufs=1) as wpool,
        tc.tile_pool(name="xc", bufs=2) as xpool,
        tc.tile_pool(name="vt", bufs=2) as vtpool,
        tc.tile_pool(name="p1q", bufs=1, space="PSUM") as p1q,
        tc.tile_pool(name="p1kv", bufs=1, space="PSUM") as p1kv,
    ):
        wq_sb = wpool.tile([128, 16 * QF], BF16, name="wq_sb")
        wk_sb = wpool.tile([128, 16 * DH], BF16, name="wk_sb")
        wv_sb = wpool.tile([128, 16 * DH], BF16, name="wv_sb")
        nc.gpsimd.dma_start(wq_sb[:, :], wqh[:, :])
        nc.sync.dma_start(wk_sb[:, :], wkh[:, :])
        nc.sync.dma_start(wv_sb[:, :], wvh[:, :])

        for lc in range(NLC1):
            x0 = xpool.tile([128, 16 * LC1], BF16, name="x0", tag="x0")
            x1 = xpool.tile([128, 16 * LC1], BF16, name="x1", tag="x1")
            nc.gpsimd.dma_start(x0[:, :], xh0[lc * 128 : (lc + 1) * 128, :])
            nc.gpsimd.dma_start(x1[:, :], xh1[lc * 128 : (lc + 1) * 128, :])
            cols = slice(lc * LC1, (lc + 1) * LC1)

            aq = [
                p1q.tile([128, LC1], F32, name=f"aq{h}", tag=f"aq{h}")
                for h in range(4)
            ]
            ak = p1kv.tile([128, LC1], F32, name="ak", tag="ak")
            av = p1kv.tile([128, LC1], F32, name="av", tag="av")
            for fb in range(16):
                st = dict(start=(fb == 0), stop=(fb == 15))
                x0f = x0[:, fb * LC1 : (fb + 1) * LC1]
                x1f = x1[:, fb * LC1 : (fb + 1) * LC1]
                for h in range(4):
                    w = wq_sb[:, fb * QF + h * DH : fb * QF + (h + 1) * DH]
                    nc.tensor.matmul(aq[h][0:64, :], w, x0f, **st)
                    nc.tensor.matmul(
                        aq[h][64:128, :], w, x1f, skip_group_check=True, **st
                    )
                wkb = wk_sb[:, fb * DH : (fb + 1) * DH]
                nc.tensor.matmul(ak[0:64, :], wkb, x0f, **st)
                nc.tensor.matmul(ak[64:128, :], wkb, x1f, skip_group_check=True, **st)
                wvb = wv_sb[:, fb * DH : (fb + 1) * DH]
                nc.tensor.matmul(av[0:64, :], wvb, x0f, **st)
                nc.tensor.matmul(av[64:128, :], wvb, x1f, skip_group_check=True, **st)

            for h in range(4):
                nc.scalar.copy(qh[h][:, cols], aq[h][:, :])
            nc.scalar.copy(kbT[:, cols], ak[:, :])
            # v needs transposing to [keys, features] for the AV matmul
            vsb = vtpool.tile([128, LC1], BF16, name="vsb", tag="vsb")
            nc.scalar.copy(vsb[:, :], av[:, :])
            for s in range(LC1 // 128):
                beta = (lc * LC1) // 128 + s
                tp = p1kv.tile([128, 128], BF16, name="tp", tag="tp", bufs=2)
                nc.tensor.matmul(
                    tp[:, :],
                    vsb[:, s * 128 : (s + 1) * 128],
                    ident[:, :],
                    is_transpose=True,
                )
                nc.scalar.copy(va[:, beta, 0:64], tp[:, 0:64])
                nc.scalar.copy(va[:, beta, 65:129], tp[:, 64:128])


def _phase2_attn(nc, tc, qh, kbT, va, bin_):
    """Transposed-scores causal attention. Head pairs share AV matmuls
    (1024-wide moving); the two batches' score matmuls run concurrently as
    row-tiled pairs."""
    with (
        tc.tile_pool(name="p2s", bufs=1, space="PSUM") as scp,
        tc.tile_pool(name="p2o", bufs=1, space="PSUM") as ovp,
        tc.tile_pool(name="pbuf", bufs=4) as pbp,
        tc.tile_pool(name="stg", bufs=3) as stp,
    ):
        for hp in range(2):
            h0, h1 = 2 * hp, 2 * hp + 1
            for tau in range(NT):
                nb = 4 * tau + 4
                qcols_full = slice(tau * LT, (tau + 1) * LT)
                oab = ovp.tile([65, 1024], F32, name="oab", tag="oab")
                obb = ovp.tile([65, 1024], F32, name="obb", tag="obb")
                for blk in range(nb):
                    dj = blk - 4 * tau  # >= 0 on the causal diagonal
                    off = max(dj, 0) * 128
                    qcols = slice(tau * LT + off, (tau + 1) * LT)
                    kc = slice(blk * 128, (blk + 1) * 128)
                    Pb = pbp.tile([128, 2048], BF16, name="Pb", tag="Pb")
                    sc0 = scp.tile([128, 1024], F32, name="sc0", tag="sc0")
                    sc1 = scp.tile([128, 1024], F32, name="sc1", tag="sc1")
                    # scores^T: row-tiled pairs (b0 rows 0:64, b1 rows 64:128)
                    nc.tensor.matmul(sc0[:, off:512], kbT[0:64, kc], qh[h0][0:64, qcols])
                    nc.tensor.matmul(
                        sc0[:, 512 + off : 1024],
                        kbT[64:128, kc],
                        qh[h0][64:128, qcols],
                        skip_group_check=True,
                    )
                    nc.tensor.matmul(sc1[:, off:512], kbT[0:64, kc], qh[h1][0:64, qcols])
                    nc.tensor.matmul(
                        sc1[:, 512 + off : 1024],
                        kbT[64:128, kc],
                        qh[h1][64:128, qcols],
                        skip_group_check=True,
                    )
                    # exp -> P block [h0b0 | h1b0 | h0b1 | h1b1]
                    if off:
                        for base in (0, 512, 1024, 1536):
                            nc.gpsimd.memset(Pb[:, base : base + off], 0.0)
                    nc.scalar.activation(Pb[:, off:512], sc0[:, off:512], Exp)
                    nc.scalar.activation(
                        Pb[:, 1024 + off : 1536], sc0[:, 512 + off : 1024], Exp
                    )
                    nc.scalar.activation(Pb[:, 512 + off : 1024], sc1[:, off:512], Exp)
                    nc.scalar.activation(
                        Pb[:, 1536 + off : 2048], sc1[:, 512 + off : 1024], Exp
                    )
                    if dj >= 0:
                        for base in (0, 512, 1024, 1536):
                            dg = Pb[:, base + off : base + off + 128]
                            nc.gpsimd.affine_select(
                                out=dg,
                                in_=dg,
                                compare_op=mybir.AluOpType.is_ge,
                                fill=0.0,
                                base=0,
                                pattern=[[1, 128]],
                                channel_multiplier=-1,
                            )
                    # AV (+denominator); psum matmul output is capped at one
                    # bank (512 f32), so each head gets its own 512-wide MM
                    st = dict(start=(blk == 0), stop=(blk == nb - 1))
                    nc.tensor.matmul(
                        oab[:, 0:512], va[:, blk, 0:65], Pb[:, 0:512], **st
                    )
                    nc.tensor.matmul(
                        oab[:, 512:1024], va[:, blk, 0:65], Pb[:, 512:1024],
                        skip_group_check=True, **st,
                    )
                    nc.tensor.matmul(
                        obb[:, 0:512], va[:, blk, 65:130], Pb[:, 1024:1536],
                        skip_group_check=True, **st,
                    )
                    nc.tensor.matmul(
                        obb[:, 512:1024], va[:, blk, 65:130], Pb[:, 1536:2048],
                        skip_group_check=True, **st,
                    )

                # stage attn rows + denominators -> A2A bounce buffer.
                # dest shard for (batch bb, seq tile tau) is 4*bb + tau;
                # rows inside shard: 65*h..65*h+64 = attn of head h, +64 denom.
                stg0 = stp.tile([128, LT], BF16, name="stg0", tag="stg0")
                stg1 = stp.tile([128, LT], BF16, name="stg1", tag="stg1")
                nc.scalar.copy(stg0[0:64, :], oab[0:64, 0:512])
                nc.scalar.copy(stg0[64:128, :], oab[0:64, 512:1024])
                nc.scalar.copy(stg1[0:64, :], obb[0:64, 0:512])
                nc.scalar.copy(stg1[64:128, :], obb[0:64, 512:1024])
                dsg = stp.tile([128, 2048], BF16, name="dsg", tag="dsg")
                nc.vector.tensor_copy(dsg[64:65, 0:512], oab[64:65, 0:512])
                nc.vector.tensor_copy(dsg[64:65, 512:1024], oab[64:65, 512:1024])
                nc.vector.tensor_copy(dsg[64:65, 1024:1536], obb[64:65, 0:512])
                nc.vector.tensor_copy(dsg[64:65, 1536:2048], obb[64:65, 512:1024])
                for bb, stg in ((0, stg0), (1, stg1)):
                    sh = SH * (4 * bb + tau)
                    nc.sync.dma_start(
                        bin_[sh + 65 * h0 : sh + 65 * h0 + 64, :], stg[0:64, :]
                    )
                    nc.sync.dma_start(
                        bin_[sh + 65 * h1 : sh + 65 * h1 + 64, :], stg[64:128, :]
                    )
                for i, (bb, hh) in enumerate(((0, h0), (0, h1), (1, h0), (1, h1))):
                    sh = SH * (4 * bb + tau)
                    r = sh + 65 * hh + 64
                    nc.sync.dma_start(
                        bin_[r : r + 1, :], dsg[64:65, 512 * i : 512 * (i + 1)]
                    )


def _phase4_oproj(nc, tc, bout, wo_sb, y):
    """Normalize (divide by softmax denominators) and run o_proj for this
    core's 512 sequence rows against the full Wo."""
    with (
        tc.tile_pool(name="an", bufs=1) as anp,
        tc.tile_pool(name="ysb", bufs=2) as yp,
        tc.tile_pool(name="p4y", bufs=2, space="PSUM") as eyp,
    ):
        ans = []
        for k in range(16):
            c, half = divmod(k, 2)
            hA, hB = 2 * half, 2 * half + 1
            au = anp.tile([128, LT], BF16, name=f"au{k}", tag=f"au{k}")
            base = SH * c
            nc.sync.dma_start(au[0:64, :], bout[base + 65 * hA : base + 65 * hA + 64, :])
            nc.sync.dma_start(
                au[64:128, :], bout[base + 65 * hB : base + 65 * hB + 64, :]
            )
            dv = anp.tile([128, LT], BF16, name="dv", tag="dv", bufs=2)
            nc.sync.dma_start(
                dv[0:64, :],
                bout[base + 65 * hA + 64 : base + 65 * hA + 65, :].partition_broadcast(
                    64
                ),
            )
            nc.sync.dma_start(
                dv[64:128, :],
                bout[base + 65 * hB + 64 : base + 65 * hB + 65, :].partition_broadcast(
                    64
                ),
            )
            rv = anp.tile([128, LT], F32, name="rv", tag="rv", bufs=2)
            nc.vector.reciprocal(rv[:, :], dv[:, :])
            an = anp.tile([128, LT], BF16, name=f"an{k}", tag=f"an{k}")
            nc.vector.tensor_mul(an[:, :], au[:, :], rv[:, :])
            ans.append(an)

        for m in range(4):
            yps = eyp.tile([128, D], F32, name="yps", tag="yps")
            for k in range(16):
                st = dict(start=(k == 0), stop=(k == 15))
                for q in range(4):
                    nc.tensor.matmul(
                        yps[:, q * 512 : (q + 1) * 512],
                        ans[k][:, m * 128 : (m + 1) * 128],
                        wo_sb[:, k * D + q * 512 : k * D + (q + 1) * 512],
                        skip_group_check=(q > 0),
                        **st,
                    )
            ysb = yp.tile([128, D], F32, name="ysb", tag="ysb")
            nc.scalar.copy(ysb[:, :], yps[:, :])
            nc.sync.dma_start(y[m * 128 : (m + 1) * 128, :], ysb[:, :])


def _get_nc():
    if "nc" not in _CACHE:
        _CACHE["nc"] = _build_nc()
    return _CACHE["nc"]


LAST_EXEC_NS = None


def _prep_x(xb):
    """x[b] [L, D] f32 -> [NLC1*128, 16*LC1] bf16 laid out so the per-chunk
    SBUF tile load is one contiguous 2D slice: row lc*128+p, col fb*LC1+t,
    value x[lc*LC1+t, fb*128+p]."""
    xT = xb.T.astype(ml_dtypes.bfloat16)  # [D, L]
    v = xT.reshape(16, 128, NLC1, LC1)  # fb, p, lc, t
    v = v.transpose(2, 1, 0, 3)  # lc, p, fb, t
    return np.ascontiguousarray(v.reshape(NLC1 * 128, 16 * LC1))


def kernel(x, Wq, Wk, Wv, Wo):
    global LAST_EXEC_NS
    x = np.asarray(x, dtype=np.float32)
    Wq = np.asarray(Wq, dtype=np.float32)
    Wk = np.asarray(Wk, dtype=np.float32)
    Wv = np.asarray(Wv, dtype=np.float32)
    Wo = np.asarray(Wo, dtype=np.float32)

    xh0 = _prep_x(x[0])
    xh1 = _prep_x(x[1])
    woh = np.ascontiguousarray(Wo.T.astype(ml_dtypes.bfloat16))

    in_maps = []
    for c in range(NCORES):
        # wq rows for this core, scaled by 1/sqrt(dh); layout [128, fb*256+h*64+i]
        wq_c = (SCALE * Wq[QF * c : QF * (c + 1), :]).astype(ml_dtypes.bfloat16)
        # wq_c[h*64+i, fb*128+p] -> wqh[p, fb*256 + h*64 + i]
        wqh = np.ascontiguousarray(
            wq_c.reshape(QF, 16, 128).transpose(2, 1, 0).reshape(128, 16 * QF)
        )
        wk_c = Wk[DH * c : DH * (c + 1), :].astype(ml_dtypes.bfloat16)
        wkh = np.ascontiguousarray(
            wk_c.reshape(DH, 16, 128).transpose(2, 1, 0).reshape(128, 16 * DH)
        )
        wv_c = Wv[DH * c : DH * (c + 1), :].astype(ml_dtypes.bfloat16)
        wvh = np.ascontiguousarray(
            wv_c.reshape(DH, 16, 128).transpose(2, 1, 0).reshape(128, 16 * DH)
        )
        in_maps.append(
            {
                "xh0": xh0,
                "xh1": xh1,
                "wqh": wqh,
                "wkh": wkh,
                "wvh": wvh,
                "woh": woh,
            }
        )

    nc = _get_nc()
    res = run_bass_kernel_spmd(nc, in_maps, core_ids=list(range(NCORES)))
    LAST_EXEC_NS = getattr(res, "exec_time_ns", None)

    out = np.empty((B, L, D), dtype=np.float32)
    for c in range(NCORES):
        b, g = divmod(c, 4)
        out[b, 512 * g : 512 * (g + 1), :] = res.results[c]["y"]
    return out


# revision 12
# speedup vs baseline: 1.7581x; 1.2879x over previous
"""GQA causal attention block (B=2, L=2048, d_model=2048, 32 Q heads / 8 KV heads)
on 8 TRN2 NeuronCores.

Sharding: 8-way tensor parallel over heads. Core c owns q-heads [4c, 4c+4) and
kv-head c for both batches. After attention, AllToAlls switch head-sharding ->
sequence-sharding (core c = batch c//4, seq block c%4 of 512); each core then
normalizes and runs o_proj against the full Wo for its 512 rows.

All matmuls run in bf16 (fp32 PSUM accumulation for AV / o_proj; scores use a
bf16 PSUM tile so the moving operand can be 1024 wide under the one-bank
output limit). Layouts:
  - qp[p] (bf16): per local head-pair a [128, 4*1024] tile: batch0 in
    partitions 0:64, batch1 in 64:128; columns tau*1024 + {h_even 512 |
    h_odd 512}.
  - kbT (bf16): [128, L]: batch0 kv-head in partitions 0:64, batch1 in
    64:128, so transposed scores for the two batches run as concurrent
    row-tiled matmuls (row groups 0-1 and 2-3).
  - va (bf16): per key-block [128, 130]: cols 0:64 = v(b0), col 64 = ones,
    cols 65:129 = v(b1), col 129 = ones. The ones column makes the AV matmul
    emit the softmax denominator for free.
  - P (bf16, rolling blocks): per key-block [128, 2048] =
    [h0b0 | h1b0 | h0b1 | h1b1] x 512 queries.

The attention loop is head-pair-major; each pair's results go through its own
AllToAll (bf16), so the first collective overlaps the second pair's compute,
and o_proj runs in two k-passes (pair 0 accumulates in PSUM then parks in
SBUF while pair 1's collective lands).
"""

import os
import sys
import math

os.environ.setdefault("MYCRO_LOCAL_CACHE", "1")
for _p in ("/opt/trn_rl_repo",):
    if os.path.isdir(_p) and _p not in sys.path:
        sys.path.insert(0, _p)

import numpy as np
import ml_dtypes

import concourse.bass as bass
import concourse.bacc as bacc
import concourse.mybir as mybir
import concourse.tile as tile
from concourse.bass_utils import run_bass_kernel_spmd
from concourse.masks import make_identity

F32 = mybir.dt.float32
BF16 = mybir.dt.bfloat16
Exp = mybir.ActivationFunctionType.Exp

D = 2048          # d_model
L = 2048          # sequence length
DH = 64           # head dim
B = 2             # batch
NCORES = 8
NH_L = 4          # local q heads per core (per batch)
QF = NH_L * DH    # 256 local q features per batch
LC1 = 512         # phase-1 token chunk
NLC1 = L // LC1   # 4
LT = 512          # attention query tile (per tau)
NT = L // LT      # 4
NB = L // 128     # 16 key blocks of 128
SH = 2 * (DH + 1)  # 130 rows per A2A shard (2 heads x (64 attn + 1 denom))
SCALE = 1.0 / math.sqrt(DH)

_CACHE = {}


def _build_nc():
    nc = bacc.Bacc(
        "TRN2",
        target_bir_lowering=False,
        debug=False,
        enable_asserts=False,
        num_devices=NCORES,
    )
    # host-prepped layouts (see kernel() below)
    xh0 = nc.dram_tensor("xh0", [NLC1 * 128, 16 * LC1], BF16, kind="ExternalInput")
    xh1 = nc.dram_tensor("xh1", [NLC1 * 128, 16 * LC1], BF16, kind="ExternalInput")
    wqh = nc.dram_tensor("wqh", [128, 16 * QF], BF16, kind="ExternalInput")
    wkvh = nc.dram_tensor("wkvh", [128, 16 * 256], BF16, kind="ExternalInput")
    woh = nc.dram_tensor("woh", [D, D], BF16, kind="ExternalInput")  # Wo.T
    y = nc.dram_tensor("y", [LT, D], F32, kind="ExternalOutput")

    with tile.TileContext(nc) as tc:
        with tc.tile_pool(name="dram", bufs=1, space="DRAM") as dram:
            bins = [
                dram.tile([NCORES * SH, LT], BF16, name=f"bin{hp}") for hp in range(2)
            ]
            bouts = [
                dram.tile([NCORES * SH, LT], BF16, name=f"bout{hp}") for hp in range(2)
            ]
            rdram = dram.tile([32, LT], BF16, name="rdram")

            with tc.tile_pool(name="const", bufs=1) as const:
                ident = const.tile([128, 128], BF16, name="ident")
                make_identity(nc, ident)

                with tc.tile_pool(name="pers", bufs=1) as pers:
                    qp = [
                        pers.tile([128, NT * 1024], BF16, name=f"qp{p}")
                        for p in range(2)
                    ]
                    kbT = pers.tile([128, L], BF16, name="kbT")
                    vaug = pers.tile([128, NB * 130], BF16, name="vaug")
                    va = vaug.rearrange("p (b c) -> p b c", c=130)
                    nc.gpsimd.memset(va[:, :, 64:65], 1.0)
                    nc.gpsimd.memset(va[:, :, 129:130], 1.0)

                    _phase1_qkv(nc, tc, xh0, xh1, wqh, wkvh, qp, kbT, va, ident)
                    with tc.tile_pool(name="wo", bufs=1) as wop:
                        wo_sb = wop.tile([128, 16 * D], BF16, name="wo_sb")
                        # prefetch the full Wo during phase 2
                        nc.gpsimd.dma_start(
                            wo_sb.rearrange("p (k d) -> p k d", d=D),
                            woh.rearrange("(k p) d -> p k d", p=128),
                        )
                        with (
                            tc.tile_pool(name="p2s", bufs=1, space="PSUM") as scp,
                            tc.tile_pool(name="p2o", bufs=1, space="PSUM") as ovp,
                            tc.tile_pool(name="pbuf", bufs=4) as pbp,
                            tc.tile_pool(name="stg", bufs=3) as stp,
                        ):
                            pools = (scp, ovp, pbp, stp)
                            for hp in range(2):
                                _attn_pair(nc, tc, pools, qp, kbT, va, bins[hp], hp)
                                nc.gpsimd.collective_compute(
                                    "AllToAll",
                                    mybir.AluOpType.bypass,
                                    ins=[bins[hp].opt()],
                                    outs=[bouts[hp].opt()],
                                    replica_groups=[list(range(NCORES))],
                                )
                        _phase4_oproj(nc, tc, bouts, rdram, wo_sb, y)
    nc.finalize()
    return nc


def _phase1_qkv(nc, tc, xh0, xh1, wqh, wkvh, qp, kbT, va, ident):
    """Projections with M=128 stationaries: q head pairs ([h_even|h_odd] out
    features) and fused k|v. The kv stationary column order is swapped for
    batch1 ([v|k]) so every psum->SBUF copy stays partition-aligned; the one
    unavoidable partition-crossing q copy per pair goes through an SBUF->SBUF
    DMA bounce."""
    with (
        tc.tile_pool(name="w1", bufs=1) as wpool,
        tc.tile_pool(name="xc", bufs=2) as xpool,
        tc.tile_pool(name="vt", bufs=2) as vtpool,
        tc.tile_pool(name="p1q", bufs=1, space="PSUM") as p1q,
        tc.tile_pool(name="p1kv", bufs=1, space="PSUM") as p1kv,
    ):
        wq_sb = wpool.tile([128, 16 * QF], BF16, name="wq_sb")
        wkv_sb = wpool.tile([128, 16 * 256], BF16, name="wkv_sb")
        nc.sync.dma_start(wq_sb[:, :], wqh[:, :])
        nc.sync.dma_start(wkv_sb[:, :], wkvh[:, :])

        for lc in range(NLC1):
            x0 = xpool.tile([128, 16 * LC1], BF16, name="x0", tag="x0")
            x1 = xpool.tile([128, 16 * LC1], BF16, name="x1", tag="x1")
            for part in range(4):
                cs = slice(part * 4 * LC1, (part + 1) * 4 * LC1)
                nc.gpsimd.dma_start(x0[:, cs], xh0[lc * 128 : (lc + 1) * 128, cs])
                nc.gpsimd.dma_start(x1[:, cs], xh1[lc * 128 : (lc + 1) * 128, cs])
            cols = slice(lc * LC1, (lc + 1) * LC1)

            aq = [
                [
                    p1q.tile([128, LC1], F32, name=f"aq{p}{b}", tag=f"aq{p}{b}")
                    for b in range(2)
                ]
                for p in range(2)
            ]
            akv = [
                p1kv.tile([128, LC1], F32, name=f"akv{b}", tag=f"akv{b}")
                for b in range(2)
            ]
            for fb in range(16):
                st = dict(start=(fb == 0), stop=(fb == 15))
                x0f = x0[:, fb * LC1 : (fb + 1) * LC1]
                x1f = x1[:, fb * LC1 : (fb + 1) * LC1]
                for p in range(2):
                    w = wq_sb[:, fb * QF + p * 128 : fb * QF + (p + 1) * 128]
                    nc.tensor.matmul(aq[p][0][:, :], w, x0f, **st)
                    nc.tensor.matmul(aq[p][1][:, :], w, x1f, **st)
                wkv0 = wkv_sb[:, fb * 256 : fb * 256 + 128]  # [k|v] for b0
                wkv1 = wkv_sb[:, fb * 256 + 128 : (fb + 1) * 256]  # [v|k] for b1
                nc.tensor.matmul(akv[0][:, :], wkv0, x0f, **st)
                nc.tensor.matmul(akv[1][:, :], wkv1, x1f, **st)

            for p in range(2):
                qc = lc * 1024
                # (h_even, b0): aligned; (h_odd, b1): aligned;
                # (h_even, b1): proven up-crossing copy;
                # (h_odd, b0): down-crossing -> DMA bounce
                nc.scalar.copy(qp[p][0:64, qc : qc + 512], aq[p][0][0:64, :])
                nc.scalar.copy(qp[p][64:128, qc : qc + 512], aq[p][1][0:64, :])
                nc.scalar.copy(
                    qp[p][64:128, qc + 512 : qc + 1024], aq[p][1][64:128, :]
                )
                tq = vtpool.tile([128, LC1], BF16, name="tq", tag=f"tq{p}")
                nc.scalar.copy(tq[64:128, :], aq[p][0][64:128, :])
                nc.sync.dma_start(qp[p][0:64, qc + 512 : qc + 1024], tq[64:128, :])
            nc.scalar.copy(kbT[0:64, cols], akv[0][0:64, :])
            nc.scalar.copy(kbT[64:128, cols], akv[1][64:128, :])
            # v -> [b0 feats | b1 feats] on partitions, then transpose
            vsb = vtpool.tile([128, LC1], BF16, name="vsb", tag="vsb")
            nc.scalar.copy(vsb[0:64, :], akv[1][0:64, :])  # b1 v (low in [v|k])
            nc.scalar.copy(vsb[64:128, :], akv[0][64:128, :])  # b0 v
            for s in range(LC1 // 128):
                beta = (lc * LC1) // 128 + s
                tp = p1kv.tile([128, 128], BF16, name="tp", tag="tp", bufs=2)
                nc.tensor.matmul(
                    tp[:, :],
                    vsb[:, s * 128 : (s + 1) * 128],
                    ident[:, :],
                    is_transpose=True,
                )
                nc.scalar.copy(va[:, beta, 0:64], tp[:, 64:128])  # b0
                nc.scalar.copy(va[:, beta, 65:129], tp[:, 0:64])  # b1


def _attn_pair(nc, tc, pools, qp, kbT, va, bin_, hp):
    """Causal attention for head pair hp (both batches)."""
    scp, ovp, pbp, stp = pools
    h0, h1 = 2 * hp, 2 * hp + 1
    for tau in range(NT):
        nb = 4 * tau + 4
        oab = ovp.tile([65, 1024], F32, name="oab", tag="oab")
        obb = ovp.tile([65, 1024], F32, name="obb", tag="obb")
        for blk in range(nb):
            dj = blk - 4 * tau  # >= 0 on the causal diagonal
            off = max(dj, 0) * 128
            kc = slice(blk * 128, (blk + 1) * 128)
            tq = tau * 1024
            Pb = pbp.tile([128, 2048], BF16, name="Pb", tag="Pb")
            sb0 = scp.tile([128, 1024], F32, name="sb0", tag="sb0")
            sb1 = scp.tile([128, 1024], F32, name="sb1", tag="sb1")
            # scores^T: row-tiled concurrent pairs (b0 rows 0:64, b1 64:128);
            # sb0 = [h0b0 | h1b0], sb1 = [h0b1 | h1b1]
            for hh, base in ((0, 0), (1, 512)):
                nc.tensor.matmul(
                    sb0[:, base + off : base + 512],
                    kbT[0:64, kc],
                    qp[hp][0:64, tq + base + off : tq + base + 512],
                    skip_group_check=(hh > 0),
                )
                nc.tensor.matmul(
                    sb1[:, base + off : base + 512],
                    kbT[64:128, kc],
                    qp[hp][64:128, tq + base + off : tq + base + 512],
                    skip_group_check=True,
                )
            # exp -> P block [h0b0 | h1b0 | h0b1 | h1b1]
            if off:
                for base in (0, 512, 1024, 1536):
                    nc.gpsimd.memset(Pb[:, base : base + off], 0.0)
                for base in (0, 512):
                    nc.scalar.activation(
                        Pb[:, base + off : base + 512], sb0[:, base + off : base + 512], Exp
                    )
                    nc.scalar.activation(
                        Pb[:, 1024 + base + off : 1024 + base + 512],
                        sb1[:, base + off : base + 512],
                        Exp,
                    )
            else:
                nc.scalar.activation(Pb[:, 0:1024], sb0[:, :], Exp)
                nc.scalar.activation(Pb[:, 1024:2048], sb1[:, :], Exp)
            if dj >= 0:
                for base in (0, 512, 1024, 1536):
                    dg = Pb[:, base + off : base + off + 128]
                    nc.gpsimd.affine_select(
                        out=dg,
                        in_=dg,
                        compare_op=mybir.AluOpType.is_ge,
                        fill=0.0,
                        base=0,
                        pattern=[[1, 128]],
                        channel_multiplier=-1,
                    )
            # AV (+denominator)
            st = dict(start=(blk == 0), stop=(blk == nb - 1))
            nc.tensor.matmul(oab[:, 0:512], va[:, blk, 0:65], Pb[:, 0:512], **st)
            nc.tensor.matmul(
                oab[:, 512:1024], va[:, blk, 0:65], Pb[:, 512:1024],
                skip_group_check=True, **st,
            )
            nc.tensor.matmul(
                obb[:, 0:512], va[:, blk, 65:130], Pb[:, 1024:1536],
                skip_group_check=True, **st,
            )
            nc.tensor.matmul(
                obb[:, 512:1024], va[:, blk, 65:130], Pb[:, 1536:2048],
                skip_group_check=True, **st,
            )

        # stage attn rows + denominators -> this pair's A2A bounce buffer.
        # dest shard for (batch bb, seq tile tau) is 4*bb + tau; rows inside
        # shard: 65*hh..65*hh+64 = attn of pair-local head hh, +64 = denom.
        for bb, src in ((0, oab), (1, obb)):
            sh = SH * (4 * bb + tau)
            for hh, base in ((0, 0), (1, 512)):
                stg = stp.tile([64, LT], BF16, name="stg", tag=f"stg{bb}{hh}")
                nc.vector.tensor_copy(stg[:, :], src[0:64, base : base + 512])
                nc.sync.dma_start(
                    bin_[sh + 65 * hh : sh + 65 * hh + 64, :], stg[:, :]
                )
        dsg = stp.tile([128, 2048], BF16, name="dsg", tag="dsg")
        nc.vector.tensor_copy(dsg[64:65, 0:512], oab[64:65, 0:512])
        nc.vector.tensor_copy(dsg[64:65, 512:1024], oab[64:65, 512:1024])
        nc.vector.tensor_copy(dsg[64:65, 1024:1536], obb[64:65, 0:512])
        nc.vector.tensor_copy(dsg[64:65, 1536:2048], obb[64:65, 512:1024])
        for i, (bb, hh) in enumerate(((0, 0), (0, 1), (1, 0), (1, 1))):
            sh = SH * (4 * bb + tau)
            r = sh + 65 * hh + 64
            nc.sync.dma_start(
                bin_[r : r + 1, :], dsg[64:65, 512 * i : 512 * (i + 1)]
            )


def _phase4_oproj(nc, tc, bouts, rdram, wo_sb, y):
    """Normalize (multiply by reciprocal softmax denominators) and run o_proj
    for this core's 512 sequence rows against the full Wo. Runs in two
    k-passes (one per A2A) so pass A overlaps the second collective."""
    with (
        tc.tile_pool(name="an", bufs=1) as anp,
        tc.tile_pool(name="ysum", bufs=1) as ysp,
        tc.tile_pool(name="ysb", bufs=2) as yp,
        tc.tile_pool(name="p4y", bufs=2, space="PSUM") as eyp,
    ):
        ans = {}
        for half in range(2):
            bo = bouts[half]
            # denominators: rows 64 + 65*i, i = 2c+hh -> one strided DMA
            dall = anp.tile([16, LT], BF16, name="dall", tag=f"dall{half}")
            nc.sync.dma_start(
                dall[:, :], bo.rearrange("(i r) c -> i r c", r=65)[:, 64, :]
            )
            rall = anp.tile([16, LT], F32, name="rall", tag=f"rall{half}")
            nc.vector.reciprocal(rall[:, :], dall[:, :])
            rbf = anp.tile([16, LT], BF16, name="rbf", tag=f"rbf{half}")
            nc.vector.tensor_copy(rbf[:, :], rall[:, :])
            nc.sync.dma_start(rdram[16 * half : 16 * (half + 1), :], rbf[:, :])
            for c in range(NCORES):
                k = 2 * c + half
                au = anp.tile([128, LT], BF16, name=f"au{k}", tag=f"au{k}")
                base = SH * c
                nc.sync.dma_start(au[0:64, :], bo[base : base + 64, :])
                nc.sync.dma_start(au[64:128, :], bo[base + 65 : base + 129, :])
                dv = anp.tile([128, LT], BF16, name="dv", tag="dv", bufs=2)
                nc.sync.dma_start(
                    dv[0:64, :],
                    rdram[16 * half + 2 * c : 16 * half + 2 * c + 1, :].partition_broadcast(64),
                )
                nc.sync.dma_start(
                    dv[64:128, :],
                    rdram[16 * half + 2 * c + 1 : 16 * half + 2 * c + 2, :].partition_broadcast(64),
                )
                an = anp.tile([128, LT], BF16, name=f"an{k}", tag=f"an{k}")
                nc.vector.tensor_mul(an[:, :], au[:, :], dv[:, :])
                ans[k] = an

            if half == 0:
                ysum = [
                    ysp.tile([128, D], F32, name=f"ysum{m}", tag=f"ysum{m}")
                    for m in range(4)
                ]
                for m in range(4):
                    yps = eyp.tile([128, D], F32, name="yps", tag="yps")
                    for ki, c in enumerate(range(NCORES)):
                        k = 2 * c
                        st = dict(start=(ki == 0), stop=(ki == NCORES - 1))
                        for q in range(4):
                            nc.tensor.matmul(
                                yps[:, q * 512 : (q + 1) * 512],
                                ans[k][:, m * 128 : (m + 1) * 128],
                                wo_sb[:, k * D + q * 512 : k * D + (q + 1) * 512],
                                skip_group_check=(q > 0),
                                **st,
                            )
                    nc.vector.tensor_copy(ysum[m][:, :], yps[:, :])
            else:
                for m in range(4):
                    yps = eyp.tile([128, D], F32, name="yps", tag="yps")
                    for ki, c in enumerate(range(NCORES)):
                        k = 2 * c + 1
                        st = dict(start=(ki == 0), stop=(ki == NCORES - 1))
                        for q in range(4):
                            nc.tensor.matmul(
                                yps[:, q * 512 : (q + 1) * 512],
                                ans[k][:, m * 128 : (m + 1) * 128],
                                wo_sb[:, k * D + q * 512 : k * D + (q + 1) * 512],
                                skip_group_check=(q > 0),
                                **st,
                            )
                    ysb = yp.tile([128, D], F32, name="ysb", tag="ysb")
                    nc.vector.tensor_add(ysb[:, :], yps[:, :], ysum[m][:, :])
                    nc.sync.dma_start(y[m * 128 : (m + 1) * 128, :], ysb[:, :])


def _get_nc():
    if "nc" not in _CACHE:
        _CACHE["nc"] = _build_nc()
    return _CACHE["nc"]


LAST_EXEC_NS = None


def _prep_x(xb):
    """x[b] [L, D] f32 -> [NLC1*128, 16*LC1] bf16 laid out so the per-chunk
    SBUF tile load is a contiguous 2D slice: row lc*128+p, col fb*LC1+t,
    value x[lc*LC1+t, fb*128+p]."""
    xT = xb.T.astype(ml_dtypes.bfloat16)  # [D, L]
    v = xT.reshape(16, 128, NLC1, LC1)  # fb, p, lc, t
    v = v.transpose(2, 1, 0, 3)  # lc, p, fb, t
    return np.ascontiguousarray(v.reshape(NLC1 * 128, 16 * LC1))


def kernel(x, Wq, Wk, Wv, Wo):
    global LAST_EXEC_NS
    x = np.asarray(x, dtype=np.float32)
    Wq = np.asarray(Wq, dtype=np.float32)
    Wk = np.asarray(Wk, dtype=np.float32)
    Wv = np.asarray(Wv, dtype=np.float32)
    Wo = np.asarray(Wo, dtype=np.float32)

    xh0 = _prep_x(x[0])
    xh1 = _prep_x(x[1])
    woh = np.ascontiguousarray(Wo.T.astype(ml_dtypes.bfloat16))

    in_maps = []
    for c in range(NCORES):
        # wq rows for this core, scaled by 1/sqrt(dh); layout [128, fb*256+o]
        wq_c = (SCALE * Wq[QF * c : QF * (c + 1), :]).astype(ml_dtypes.bfloat16)
        wqh = np.ascontiguousarray(
            wq_c.reshape(QF, 16, 128).transpose(2, 1, 0).reshape(128, 16 * QF)
        )
        wk_c = Wk[DH * c : DH * (c + 1), :].astype(ml_dtypes.bfloat16)
        wkh = wk_c.reshape(DH, 16, 128).transpose(2, 1, 0)  # [128, 16, 64]
        wv_c = Wv[DH * c : DH * (c + 1), :].astype(ml_dtypes.bfloat16)
        wvh = wv_c.reshape(DH, 16, 128).transpose(2, 1, 0)
        # per fb: [k|v] for batch0, then [v|k] for batch1
        wkvh = np.empty((128, 16, 256), dtype=ml_dtypes.bfloat16)
        wkvh[:, :, 0:64] = wkh
        wkvh[:, :, 64:128] = wvh
        wkvh[:, :, 128:192] = wvh
        wkvh[:, :, 192:256] = wkh
        wkvh = np.ascontiguousarray(wkvh.reshape(128, 16 * 256))
        in_maps.append(
            {"xh0": xh0, "xh1": xh1, "wqh": wqh, "wkvh": wkvh, "woh": woh}
        )

    nc = _get_nc()
    res = run_bass_kernel_spmd(nc, in_maps, core_ids=list(range(NCORES)))
    LAST_EXEC_NS = getattr(res, "exec_time_ns", None)

    out = np.empty((B, L, D), dtype=np.float32)
    for c in range(NCORES):
        b, g = divmod(c, 4)
        out[b, 512 * g : 512 * (g + 1), :] = res.results[c]["y"]
    return out
